# revision 22
# baseline (speedup 1.0000x reference)
"""Trainium2 Bass kernel for ExtractorLoss (PSD SNR loss).

loss = -mean_b( 10*log10( (mean wanted psd) / (mean unwanted psd) ) )
with psd[b,g] = (x @ cos_g)^2 + (x @ sin_g)^2 over a 201-bin frequency grid.

Math: grid frequencies are g/1800 cycles/sample (g = grid_bpm in 40..240,
fs = 30 Hz), so the DFT basis has period 1800 over t and half-period sign
symmetry: cos(2*pi*g*(tau+900j)/1800) = (-1)^{gj} cos(2*pi*g*tau/1800).
Folding the four 900-sample segments of x on host (plain sum for even g,
alternating sum for odd g) shrinks the GEMM contraction from 3600 to 900
with two parity classes — 4x less PE work and x DMA than the naive GEMM.

Sharding: data-parallel over batch across 8 NeuronCores (512 rows each).
Host packs, per core, a [900, 1428] bf16 tensor: [xeT(512) | xoT(512) |
basis_e(202) | basis_o(202)] so each contraction k-tile is one large DMA.
Odd-parity bins (100) are zero-padded to 101 so even/odd blocks align.

Raw Bacc kernel (no TileContext — avoids its ~12us semaphore-clear +
barrier tail): the Sync engine streams the 8 k-tile DMAs with a window-2
issue throttle (so the first tile completes early instead of round-robin
sharing bandwidth with the whole stream), PE accumulates both parity
GEMMs into 8 PSUM banks, then a per-row-tile interleaved epilogue runs
Square on ScalarE and fused multiply-reduce (tensor_tensor_reduce) on
DVE. The tiny log/mean runs on host in float64.
"""

import functools
import sys

import numpy as np
import ml_dtypes

if "/opt/trn_rl_repo" not in sys.path:
    sys.path.insert(0, "/opt/trn_rl_repo")

# Problem constants (fixed by the problem spec).
B, T, NG = 4096, 3600, 201
NCORES = 8
BS = B // NCORES          # 512 batch rows per core
MT = BS // 128            # 4 output partition tiles per core
TF = T // 4               # 900 folded contraction length
KT = 128
NK = (TF + KT - 1) // KT  # 8 k-tiles (7x128 + 1x4)
NGE = 101                 # even-parity bins (grid 40,42..240)
NGO = 100                 # odd-parity bins (grid 41,43..239)
NGP = 101                 # odd padded to 101
NBH = 2 * NGP             # 202 basis cols per parity (cos|sin)
XCOL = 2 * BS             # 1024 x cols (xe | xo)
PCOL = XCOL + 2 * NBH     # 1428 packed cols per k row

BF16 = ml_dtypes.bfloat16


@functools.lru_cache(maxsize=1)
def _build_program():
    import concourse.bacc as bacc
    import concourse.mybir as mybir
    from contextlib import ExitStack

    f32 = mybir.dt.float32
    bf16 = mybir.dt.bfloat16

    # Bacc (not raw Bass): its finalize() legalizes multi-wait instructions
    # into event-semaphore chains — walrus allows only 1 wait per inst.
    nc = bacc.Bacc()
    xb = nc.declare_dram_parameter("xb", [TF, PCOL], bf16, isOutput=False)
    maskd = nc.declare_dram_parameter("mask", [BS, NBH], bf16, isOutput=False)
    outd = nc.declare_dram_parameter("out", [BS, 2], f32, isOutput=True)

    ksizes = []
    off = 0
    while off < TF:
        sz = min(KT, TF - off)
        ksizes.append((off, sz))
        off += sz

    with ExitStack() as ctx:
        xsb = ctx.enter_context(nc.sbuf_tensor("xsb", [128, NK, PCOL], bf16))
        masksb = ctx.enter_context(nc.sbuf_tensor("masksb", [128, MT, NBH], bf16))
        sq = ctx.enter_context(nc.sbuf_tensor("sq", [128, MT, 2, NBH], bf16))
        msq = ctx.enter_context(nc.sbuf_tensor("msq", [128, MT, 2, NBH], bf16))
        acca = ctx.enter_context(nc.sbuf_tensor("acca", [128, MT], f32))
        accb = ctx.enter_context(nc.sbuf_tensor("accb", [128, MT], f32))
        outsb = ctx.enter_context(nc.sbuf_tensor("outsb", [128, MT, 2], f32))
        ps = ctx.enter_context(nc.psum_tensor("ps", [128, MT, 2, 512], f32))

        dsems = [
            ctx.enter_context(nc.semaphore(f"dsem{k}")) for k in range(NK)
        ]
        msem = ctx.enter_context(nc.semaphore("msem"))
        pesem = ctx.enter_context(nc.semaphore("pesem"))
        actsem = ctx.enter_context(nc.semaphore("actsem"))
        dvesem = ctx.enter_context(nc.semaphore("dvesem"))
        dv2 = ctx.enter_context(nc.semaphore("dv2"))
        osem = ctx.enter_context(nc.semaphore("osem"))

        # Raw semaphores are not cleared on allocation; zero them before any
        # engine waits (runs in the preamble block, then a full barrier).
        allsems = dsems + [msem, pesem, actsem, dvesem, osem]
        nums = sorted(sh.num for sh in allsems)
        lo = 0
        while lo < len(nums):
            hi = lo
            while hi + 1 < len(nums) and nums[hi + 1] == nums[hi] + 1:
                hi += 1
            nc.gpsimd.sem_clear(range(nums[lo], nums[hi] + 1))
            lo = hi + 1
        nc.all_engine_barrier()

        block = ctx.enter_context(nc.Block())

        @block.sync
        def _(sync):
            # Window-2 issue throttle: at most 2 x-transfers in flight, so
            # the dynamic HW queue can't round-robin the whole stream and
            # defer the first completion.
            for k, (off, sz) in enumerate(ksizes):
                if k >= 2:
                    sync.wait_ge(dsems[k - 2], 16)
                nc.sync.dma_start(
                    out=xsb[:sz, k, :], in_=xb[off : off + sz, :]
                ).then_inc(dsems[k], 16)
            nc.sync.dma_start(
                out=masksb[:], in_=maskd.rearrange("(m p) g -> p m g", p=128)
            ).then_inc(msem, 16)
            sync.wait_ge(dvesem, MT)
            nc.sync.dma_start(
                out=outd.rearrange("(m p) c -> p m c", p=128),
                in_=outsb[:],
            ).then_inc(osem, 16)
            sync.wait_ge(osem, 16)

        @block.tensor
        def _(tensor):
            last = None
            for k, (off, sz) in enumerate(ksizes):
                tensor.wait_ge(dsems[k], 16)
                start, stop = (k == 0), (k == NK - 1)
                for m in range(MT):
                    last = nc.tensor.matmul(
                        ps[:, m, 0, 0:NBH],
                        lhsT=xsb[:sz, k, m * 128 : (m + 1) * 128],
                        rhs=xsb[:sz, k, XCOL : XCOL + NBH],
                        start=start,
                        stop=stop,
                    )
                    last = nc.tensor.matmul(
                        ps[:, m, 1, 0:NBH],
                        lhsT=xsb[:sz, k, BS + m * 128 : BS + (m + 1) * 128],
                        rhs=xsb[:sz, k, XCOL + NBH : XCOL + 2 * NBH],
                        start=start,
                        stop=stop,
                    )
            # Matmuls complete in program order; one inc on the last is sound.
            last.then_inc(pesem, 1)

        @block.scalar
        def _(scalar):
            scalar.wait_ge(pesem, 1)
            for m in range(MT):
                # sq[:,m,0] <- cos^2 blocks, acc_a = their per-row sum
                nc.scalar.activation(
                    sq[:, m, 0].rearrange("p (two x) -> p two x", two=2),
                    ps[:, m, :, 0:NGP],
                    mybir.ActivationFunctionType.Square,
                    accum_out=acca[:, m : m + 1],
                )
                nc.scalar.activation(
                    sq[:, m, 1].rearrange("p (two x) -> p two x", two=2),
                    ps[:, m, :, NGP:NBH],
                    mybir.ActivationFunctionType.Square,
                    accum_out=accb[:, m : m + 1],
                ).then_inc(actsem, 1)

        @block.vector
        def _(vector):
            vector.wait_ge(msem, 16)
            # masked squares (read only ACT/DMA data -> no intra-DVE RAW)
            last = None
            for m in range(MT):
                vector.wait_ge(actsem, m + 1)
                for par in range(2):
                    last = nc.vector.tensor_mul(
                        msq[:, m, par], sq[:, m, par], masksb[:, m]
                    )
            # single pipeline-drain handshake before consuming DVE outputs
            last.then_inc(dv2, 1)
            vector.wait_ge(dv2, 1)
            for m in range(MT):
                # total_m = acc_a + acc_b (ScalarE accumulator outputs)
                nc.vector.tensor_add(
                    outsb[:, m, 0:1], acca[:, m : m + 1], accb[:, m : m + 1]
                )
                # wanted_m = sum over both parities of mask*squares
                nc.vector.tensor_reduce(
                    outsb[:, m, 1:2], msq[:, m], axis=mybir.AxisListType.XY,
                    op=mybir.AluOpType.add,
                ).then_inc(dvesem, 1)

    # Run Bacc's compile passes (register allocation, 1-wait legalization
    # via event-semaphore chains) — the PJRT exec path doesn't finalize.
    nc.finalize()
    return nc


def _host_prep(x, f_true_bpm, fs, delta_bpm, sampling_bpm, fmin_bpm, fmax_bpm):
    fs = int(fs)
    delta = int(delta_bpm)
    samp = int(sampling_bpm)
    fmin = int(fmin_bpm)
    fmax = int(fmax_bpm)

    n_grid = (fmax - fmin) // samp + 1
    assert n_grid == NG and fs == 30 and samp == 1, (n_grid, fs, samp)
    grid_bpm = fmin + samp * np.arange(n_grid, dtype=np.int64)
    ge = grid_bpm[grid_bpm % 2 == 0]  # 101 even bins
    go = grid_bpm[grid_bpm % 2 == 1]  # 100 odd bins

    # Folded basis over tau in [0, 900): theta = 2*pi*g*tau/1800.
    tau = np.arange(TF, dtype=np.float64)
    thE = 2.0 * np.pi * ge[:, None] * tau[None, :] / 1800.0  # [101, 900]
    thO = 2.0 * np.pi * go[:, None] * tau[None, :] / 1800.0  # [100, 900]
    basis = np.zeros((TF, 2 * NBH), dtype=BF16)
    basis[:, 0:NGE] = np.cos(thE).T.astype(BF16)
    basis[:, NGP : NGP + NGE] = np.sin(thE).T.astype(BF16)
    basis[:, NBH : NBH + NGO] = np.cos(thO).T.astype(BF16)
    basis[:, NBH + NGP : NBH + NGP + NGO] = np.sin(thO).T.astype(BF16)

    # Fold x: 4 segments of 900; even g sums plain, odd g alternates.
    s = x.astype(np.float64).reshape(B, 4, TF)
    xe = (s[:, 0] + s[:, 1] + s[:, 2] + s[:, 3]).astype(BF16)  # [B, 900]
    xo = (s[:, 0] - s[:, 1] + s[:, 2] - s[:, 3]).astype(BF16)

    # Wanted-band masks in [maskE(101) | maskO(100) pad] layout.
    f64 = f_true_bpm.astype(np.int64)
    mask = np.zeros((B, NBH), dtype=BF16)
    mask[:, 0:NGE] = np.abs(ge[None, :] - f64[:, None]) <= delta
    mask[:, NGP : NGP + NGO] = np.abs(go[None, :] - f64[:, None]) <= delta

    in_maps = []
    for c in range(NCORES):
        sl = slice(c * BS, (c + 1) * BS)
        xbp = np.empty((TF, PCOL), dtype=BF16)
        xbp[:, 0:BS] = xe[sl].T
        xbp[:, BS:XCOL] = xo[sl].T
        xbp[:, XCOL:] = basis
        in_maps.append(
            {"xb": xbp, "mask": np.ascontiguousarray(mask[sl])}
        )

    n_wanted = 2 * delta // samp + 1
    n_unwanted = n_grid - n_wanted
    return in_maps, n_wanted, n_unwanted


def _finalize(outs, n_wanted, n_unwanted):
    # outs: list of [BS, 2] fp32 per core with (total_sum, wanted_sum) rows.
    full = np.concatenate(outs, axis=0).astype(np.float64)  # [B, 2]
    total, wanted = full[:, 0], full[:, 1]
    term1 = wanted / n_wanted
    term2 = (total - wanted) / n_unwanted
    snr = 10.0 * np.log10(term1 / term2)
    return np.array(-snr.mean(), dtype=np.float32)


def kernel(x, f_true_bpm, fs, delta_bpm, sampling_bpm, fmin_bpm, fmax_bpm):
    from concourse.bass_utils import run_bass_kernel_spmd

    x = np.asarray(x, dtype=np.float32)
    f_true_bpm = np.asarray(f_true_bpm)
    in_maps, n_wanted, n_unwanted = _host_prep(
        x, f_true_bpm, fs, delta_bpm, sampling_bpm, fmin_bpm, fmax_bpm
    )
    nc = _build_program()
    res = run_bass_kernel_spmd(nc, in_maps, core_ids=list(range(NCORES)))
    outs = [r["out"] for r in res.results]
    return _finalize(outs, n_wanted, n_unwanted)


# revision 25
# speedup vs baseline: 1.2095x; 1.2095x over previous
"""Trainium2 Bass kernel for ExtractorLoss (PSD SNR loss).

loss = -mean_b( 10*log10( (mean wanted psd) / (mean unwanted psd) ) )
with psd[b,g] = (x @ cos_g)^2 + (x @ sin_g)^2 over a 201-bin frequency grid.

Math: grid frequencies are g/1800 cycles/sample (g = grid_bpm in 40..240,
fs = 30 Hz), so the DFT basis has period 1800 over t and half-period sign
symmetry: cos(2*pi*g*(tau+900j)/1800) = (-1)^{gj} cos(2*pi*g*tau/1800).
Folding the four 900-sample segments of x on host (plain sum for even g,
alternating sum for odd g) shrinks the GEMM contraction from 3600 to 900
with two parity classes — 4x less PE work and x DMA than the naive GEMM.

Sharding: data-parallel over batch across 8 NeuronCores (512 rows each).
Host packs, per core, a [900, 1428] bf16 tensor: [xeT(512) | xoT(512) |
basis_e(202) | basis_o(202)] so each contraction k-tile is one large DMA.
Odd-parity bins (100) are zero-padded to 101 so even/odd blocks align.

Raw Bacc kernel (no TileContext — avoids its ~12us semaphore-clear +
barrier tail): the Sync engine streams the 8 k-tile DMAs with a window-2
issue throttle (so the first tile completes early instead of round-robin
sharing bandwidth with the whole stream), PE accumulates both parity
GEMMs into 8 PSUM banks, then a per-row-tile interleaved epilogue runs
Square on ScalarE and fused multiply-reduce (tensor_tensor_reduce) on
DVE. The tiny log/mean runs on host in float64.
"""

import functools
import sys

import numpy as np
import ml_dtypes

if "/opt/trn_rl_repo" not in sys.path:
    sys.path.insert(0, "/opt/trn_rl_repo")

# Problem constants (fixed by the problem spec).
B, T, NG = 4096, 3600, 201
NCORES = 8
BS = B // NCORES          # 512 batch rows per core
MT = BS // 128            # 4 output partition tiles per core
TF = T // 4               # 900 folded contraction length
KT = 128
NK = (TF + KT - 1) // KT  # 8 k-tiles (7x128 + 1x4)
NGE = 101                 # even-parity bins (grid 40,42..240)
NGO = 100                 # odd-parity bins (grid 41,43..239)
NGP = 101                 # odd padded to 101
NBH = 2 * NGP             # 202 basis cols per parity (cos|sin)
XCOL = 2 * BS             # 1024 x cols (xe | xo)
PCOL = XCOL + 2 * NBH     # 1428 packed cols per k row

BF16 = ml_dtypes.bfloat16


@functools.lru_cache(maxsize=1)
def _build_program():
    import concourse.bacc as bacc
    import concourse.mybir as mybir
    from contextlib import ExitStack

    f32 = mybir.dt.float32
    bf16 = mybir.dt.bfloat16

    # Bacc (not raw Bass): its finalize() legalizes multi-wait instructions
    # into event-semaphore chains — walrus allows only 1 wait per inst.
    nc = bacc.Bacc()
    xb = nc.declare_dram_parameter("xb", [TF, PCOL], bf16, isOutput=False)
    maskd = nc.declare_dram_parameter("mask", [BS, 2 * NBH], bf16, isOutput=False)
    outd = nc.declare_dram_parameter("out", [BS, 2], f32, isOutput=True)

    ksizes = []
    off = 0
    while off < TF:
        sz = min(KT, TF - off)
        ksizes.append((off, sz))
        off += sz

    with ExitStack() as ctx:
        xsb = ctx.enter_context(nc.sbuf_tensor("xsb", [128, NK, PCOL], bf16))
        masksb = ctx.enter_context(nc.sbuf_tensor("masksb", [128, MT, 2, NBH], bf16))
        sq = ctx.enter_context(nc.sbuf_tensor("sq", [128, MT, 2, NBH], bf16))
        msq = ctx.enter_context(nc.sbuf_tensor("msq", [128, MT, 2, NBH], bf16))
        outsb = ctx.enter_context(nc.sbuf_tensor("outsb", [128, MT, 2], f32))
        ps = ctx.enter_context(nc.psum_tensor("ps", [128, MT, 2, 512], f32))

        dsems = [
            ctx.enter_context(nc.semaphore(f"dsem{k}")) for k in range(NK)
        ]
        msem = ctx.enter_context(nc.semaphore("msem"))
        pesem = ctx.enter_context(nc.semaphore("pesem"))
        actsem = ctx.enter_context(nc.semaphore("actsem"))
        dvesem = ctx.enter_context(nc.semaphore("dvesem"))
        dv2 = ctx.enter_context(nc.semaphore("dv2"))
        osem = ctx.enter_context(nc.semaphore("osem"))

        # Raw semaphores are not cleared on allocation; zero them before any
        # engine waits (runs in the preamble block, then a full barrier).
        allsems = dsems + [msem, pesem, actsem, dvesem, osem]
        nums = sorted(sh.num for sh in allsems)
        lo = 0
        while lo < len(nums):
            hi = lo
            while hi + 1 < len(nums) and nums[hi + 1] == nums[hi] + 1:
                hi += 1
            nc.gpsimd.sem_clear(range(nums[lo], nums[hi] + 1))
            lo = hi + 1
        nc.all_engine_barrier()

        block = ctx.enter_context(nc.Block())

        @block.sync
        def _(sync):
            for k, (off, sz) in enumerate(ksizes):
                nc.sync.dma_start(
                    out=xsb[:sz, k, :], in_=xb[off : off + sz, :]
                ).then_inc(dsems[k], 16)
            nc.sync.dma_start(
                out=masksb[:], in_=maskd.rearrange("(m p) g -> p m g", p=128)
            ).then_inc(msem, 16)
            sync.wait_ge(dvesem, 2)
            nc.sync.dma_start(
                out=outd.rearrange("(m p) c -> p m c", p=128),
                in_=outsb[:],
            ).then_inc(osem, 16)
            sync.wait_ge(osem, 16)

        @block.tensor
        def _(tensor):
            last = None
            for k, (off, sz) in enumerate(ksizes):
                tensor.wait_ge(dsems[k], 16)
                start, stop = (k == 0), (k == NK - 1)
                for m in range(MT):
                    last = nc.tensor.matmul(
                        ps[:, m, 0, 0:NBH],
                        lhsT=xsb[:sz, k, m * 128 : (m + 1) * 128],
                        rhs=xsb[:sz, k, XCOL : XCOL + NBH],
                        start=start,
                        stop=stop,
                    )
                    last = nc.tensor.matmul(
                        ps[:, m, 1, 0:NBH],
                        lhsT=xsb[:sz, k, BS + m * 128 : BS + (m + 1) * 128],
                        rhs=xsb[:sz, k, XCOL + NBH : XCOL + 2 * NBH],
                        start=start,
                        stop=stop,
                    )
            # Matmuls complete in program order; one inc on the last is sound.
            last.then_inc(pesem, 1)

        @block.scalar
        def _(scalar):
            scalar.wait_ge(pesem, 1)
            # cos^2 of all 8 banks -> sq[:, :, 0, :] (as [128, 4, 2, 101])
            nc.scalar.activation(
                sq[:, :, 0, :].rearrange("p m (two x) -> p m two x", two=2),
                ps[:, :, :, 0:NGP],
                mybir.ActivationFunctionType.Square,
            ).then_inc(actsem, 1)
            nc.scalar.activation(
                sq[:, :, 1, :].rearrange("p m (two x) -> p m two x", two=2),
                ps[:, :, :, NGP:NBH],
                mybir.ActivationFunctionType.Square,
            ).then_inc(actsem, 1)

        @block.vector
        def _(vector):
            vector.wait_ge(msem, 16)
            # masked squares per parity block (read only ACT/DMA data)
            vector.wait_ge(actsem, 1)
            nc.vector.tensor_mul(
                msq[:, :, 0], sq[:, :, 0], masksb[:, :, 0]
            )
            vector.wait_ge(actsem, 2)
            last_mul = nc.vector.tensor_mul(
                msq[:, :, 1], sq[:, :, 1], masksb[:, :, 1]
            )
            # totals read only ACT data -> safe before the drain
            nc.vector.tensor_reduce(
                outsb[:, :, 0],
                sq[:].rearrange("p m two x -> p m (two x)"),
                axis=mybir.AxisListType.X,
                op=mybir.AluOpType.add,
            ).then_inc(dvesem, 1)
            # drain DVE pipeline before reading its own msq outputs
            last_mul.then_inc(dv2, 1)
            vector.wait_ge(dv2, 1)
            nc.vector.tensor_reduce(
                outsb[:, :, 1],
                msq[:].rearrange("p m two x -> p m (two x)"),
                axis=mybir.AxisListType.X,
                op=mybir.AluOpType.add,
            ).then_inc(dvesem, 1)

    # Run Bacc's compile passes (register allocation, 1-wait legalization
    # via event-semaphore chains) — the PJRT exec path doesn't finalize.
    nc.finalize()
    return nc


def _host_prep(x, f_true_bpm, fs, delta_bpm, sampling_bpm, fmin_bpm, fmax_bpm):
    fs = int(fs)
    delta = int(delta_bpm)
    samp = int(sampling_bpm)
    fmin = int(fmin_bpm)
    fmax = int(fmax_bpm)

    n_grid = (fmax - fmin) // samp + 1
    assert n_grid == NG and fs == 30 and samp == 1, (n_grid, fs, samp)
    grid_bpm = fmin + samp * np.arange(n_grid, dtype=np.int64)
    ge = grid_bpm[grid_bpm % 2 == 0]  # 101 even bins
    go = grid_bpm[grid_bpm % 2 == 1]  # 100 odd bins

    # Folded basis over tau in [0, 900): theta = 2*pi*g*tau/1800.
    tau = np.arange(TF, dtype=np.float64)
    thE = 2.0 * np.pi * ge[:, None] * tau[None, :] / 1800.0  # [101, 900]
    thO = 2.0 * np.pi * go[:, None] * tau[None, :] / 1800.0  # [100, 900]
    basis = np.zeros((TF, 2 * NBH), dtype=BF16)
    basis[:, 0:NGE] = np.cos(thE).T.astype(BF16)
    basis[:, NGP : NGP + NGE] = np.sin(thE).T.astype(BF16)
    basis[:, NBH : NBH + NGO] = np.cos(thO).T.astype(BF16)
    basis[:, NBH + NGP : NBH + NGP + NGO] = np.sin(thO).T.astype(BF16)

    # Fold x: 4 segments of 900; even g sums plain, odd g alternates.
    s = x.astype(np.float64).reshape(B, 4, TF)
    xe = (s[:, 0] + s[:, 1] + s[:, 2] + s[:, 3]).astype(BF16)  # [B, 900]
    xo = (s[:, 0] - s[:, 1] + s[:, 2] - s[:, 3]).astype(BF16)

    # Wanted-band masks in [maskE(101) | maskO(100) pad] layout, doubled so
    # one elementwise multiply covers both (cos^2, sin^2) blocks.
    f64 = f_true_bpm.astype(np.int64)
    mask = np.zeros((B, 2, NBH), dtype=BF16)
    mask[:, 0, 0:NGE] = np.abs(ge[None, :] - f64[:, None]) <= delta
    mask[:, 0, NGP : NGP + NGO] = np.abs(go[None, :] - f64[:, None]) <= delta
    mask[:, 1, :] = mask[:, 0, :]
    mask = mask.reshape(B, 2 * NBH)

    in_maps = []
    for c in range(NCORES):
        sl = slice(c * BS, (c + 1) * BS)
        xbp = np.empty((TF, PCOL), dtype=BF16)
        xbp[:, 0:BS] = xe[sl].T
        xbp[:, BS:XCOL] = xo[sl].T
        xbp[:, XCOL:] = basis
        in_maps.append(
            {"xb": xbp, "mask": np.ascontiguousarray(mask[sl])}
        )

    n_wanted = 2 * delta // samp + 1
    n_unwanted = n_grid - n_wanted
    return in_maps, n_wanted, n_unwanted


def _finalize(outs, n_wanted, n_unwanted):
    # outs: list of [BS, 2] fp32 per core with (total_sum, wanted_sum) rows.
    full = np.concatenate(outs, axis=0).astype(np.float64)  # [B, 2]
    total, wanted = full[:, 0], full[:, 1]
    term1 = wanted / n_wanted
    term2 = (total - wanted) / n_unwanted
    snr = 10.0 * np.log10(term1 / term2)
    return np.array(-snr.mean(), dtype=np.float32)


def kernel(x, f_true_bpm, fs, delta_bpm, sampling_bpm, fmin_bpm, fmax_bpm):
    from concourse.bass_utils import run_bass_kernel_spmd

    x = np.asarray(x, dtype=np.float32)
    f_true_bpm = np.asarray(f_true_bpm)
    in_maps, n_wanted, n_unwanted = _host_prep(
        x, f_true_bpm, fs, delta_bpm, sampling_bpm, fmin_bpm, fmax_bpm
    )
    nc = _build_program()
    res = run_bass_kernel_spmd(nc, in_maps, core_ids=list(range(NCORES)))
    outs = [r["out"] for r in res.results]
    return _finalize(outs, n_wanted, n_unwanted)


# revision 26
# speedup vs baseline: 1.2108x; 1.0010x over previous
"""Trainium2 Bass kernel for ExtractorLoss (PSD SNR loss).

loss = -mean_b( 10*log10( (mean wanted psd) / (mean unwanted psd) ) )
with psd[b,g] = (x @ cos_g)^2 + (x @ sin_g)^2 over a 201-bin frequency grid.

Math: grid frequencies are g/1800 cycles/sample (g = grid_bpm in 40..240,
fs = 30 Hz), so the DFT basis has period 1800 over t and half-period sign
symmetry: cos(2*pi*g*(tau+900j)/1800) = (-1)^{gj} cos(2*pi*g*tau/1800).
Folding the four 900-sample segments of x on host (plain sum for even g,
alternating sum for odd g) shrinks the GEMM contraction from 3600 to 900
with two parity classes — 4x less PE work and x DMA than the naive GEMM.

Sharding: data-parallel over batch across 8 NeuronCores (512 rows each).
Host packs, per core, a [900, 1428] bf16 tensor: [xeT(512) | xoT(512) |
basis_e(202) | basis_o(202)] so each contraction k-tile is one large DMA.
Odd-parity bins (100) are zero-padded to 101 so even/odd blocks align.

Raw Bacc kernel (no TileContext — avoids its ~12us semaphore-clear +
barrier tail): the Sync engine streams the 8 k-tile DMAs with a window-2
issue throttle (so the first tile completes early instead of round-robin
sharing bandwidth with the whole stream), PE accumulates both parity
GEMMs into 8 PSUM banks, then a per-row-tile interleaved epilogue runs
Square on ScalarE and fused multiply-reduce (tensor_tensor_reduce) on
DVE. The tiny log/mean runs on host in float64.
"""

import functools
import sys

import numpy as np
import ml_dtypes

if "/opt/trn_rl_repo" not in sys.path:
    sys.path.insert(0, "/opt/trn_rl_repo")

# Problem constants (fixed by the problem spec).
B, T, NG = 4096, 3600, 201
NCORES = 8
BS = B // NCORES          # 512 batch rows per core
MT = BS // 128            # 4 output partition tiles per core
TF = T // 4               # 900 folded contraction length
KT = 128
NK = (TF + KT - 1) // KT  # 8 k-tiles (7x128 + 1x4)
NGE = 101                 # even-parity bins (grid 40,42..240)
NGO = 100                 # odd-parity bins (grid 41,43..239)
NGP = 101                 # odd padded to 101
NBH = 2 * NGP             # 202 basis cols per parity (cos|sin)
XCOL = 2 * BS             # 1024 x cols (xe | xo)
PCOL = XCOL + 2 * NBH     # 1428 packed cols per k row

BF16 = ml_dtypes.bfloat16


@functools.lru_cache(maxsize=1)
def _build_program():
    import concourse.bacc as bacc
    import concourse.mybir as mybir
    from contextlib import ExitStack

    f32 = mybir.dt.float32
    bf16 = mybir.dt.bfloat16

    # Bacc (not raw Bass): its finalize() legalizes multi-wait instructions
    # into event-semaphore chains — walrus allows only 1 wait per inst.
    nc = bacc.Bacc()
    xb = nc.declare_dram_parameter("xb", [TF, PCOL], bf16, isOutput=False)
    maskd = nc.declare_dram_parameter("mask", [BS, 2 * NBH], bf16, isOutput=False)
    outd = nc.declare_dram_parameter("out", [BS, 2], f32, isOutput=True)

    ksizes = []
    off = 0
    while off < TF:
        sz = min(KT, TF - off)
        ksizes.append((off, sz))
        off += sz

    with ExitStack() as ctx:
        xsb = ctx.enter_context(nc.sbuf_tensor("xsb", [128, NK, PCOL], bf16))
        masksb = ctx.enter_context(nc.sbuf_tensor("masksb", [128, MT, 2, NBH], bf16))
        sq = ctx.enter_context(nc.sbuf_tensor("sq", [128, MT, 2, NBH], f32))
        msq = ctx.enter_context(nc.sbuf_tensor("msq", [128, MT, 2, NBH], f32))
        outsb = ctx.enter_context(nc.sbuf_tensor("outsb", [128, MT, 2], f32))
        ps = ctx.enter_context(nc.psum_tensor("ps", [128, MT, 2, 512], f32))

        dsems = [
            ctx.enter_context(nc.semaphore(f"dsem{k}")) for k in range(NK)
        ]
        msem = ctx.enter_context(nc.semaphore("msem"))
        pesem = ctx.enter_context(nc.semaphore("pesem"))
        actsem = ctx.enter_context(nc.semaphore("actsem"))
        dvesem = ctx.enter_context(nc.semaphore("dvesem"))
        dv2 = ctx.enter_context(nc.semaphore("dv2"))
        osem = ctx.enter_context(nc.semaphore("osem"))

        # Raw semaphores are not cleared on allocation; zero them before any
        # engine waits (runs in the preamble block, then a full barrier).
        allsems = dsems + [msem, pesem, actsem, dvesem, osem]
        nums = sorted(sh.num for sh in allsems)
        lo = 0
        while lo < len(nums):
            hi = lo
            while hi + 1 < len(nums) and nums[hi + 1] == nums[hi] + 1:
                hi += 1
            nc.gpsimd.sem_clear(range(nums[lo], nums[hi] + 1))
            lo = hi + 1
        nc.all_engine_barrier()

        block = ctx.enter_context(nc.Block())

        @block.sync
        def _(sync):
            for k, (off, sz) in enumerate(ksizes):
                nc.sync.dma_start(
                    out=xsb[:sz, k, :], in_=xb[off : off + sz, :]
                ).then_inc(dsems[k], 16)
            nc.sync.dma_start(
                out=masksb[:], in_=maskd.rearrange("(m p) g -> p m g", p=128)
            ).then_inc(msem, 16)
            sync.wait_ge(dvesem, 2)
            nc.sync.dma_start(
                out=outd.rearrange("(m p) c -> p m c", p=128),
                in_=outsb[:],
            ).then_inc(osem, 16)
            sync.wait_ge(osem, 16)

        @block.tensor
        def _(tensor):
            last = None
            for k, (off, sz) in enumerate(ksizes):
                tensor.wait_ge(dsems[k], 16)
                start, stop = (k == 0), (k == NK - 1)
                for m in range(MT):
                    last = nc.tensor.matmul(
                        ps[:, m, 0, 0:NBH],
                        lhsT=xsb[:sz, k, m * 128 : (m + 1) * 128],
                        rhs=xsb[:sz, k, XCOL : XCOL + NBH],
                        start=start,
                        stop=stop,
                    )
                    last = nc.tensor.matmul(
                        ps[:, m, 1, 0:NBH],
                        lhsT=xsb[:sz, k, BS + m * 128 : BS + (m + 1) * 128],
                        rhs=xsb[:sz, k, XCOL + NBH : XCOL + 2 * NBH],
                        start=start,
                        stop=stop,
                    )
            # Matmuls complete in program order; one inc on the last is sound.
            last.then_inc(pesem, 1)

        @block.scalar
        def _(scalar):
            scalar.wait_ge(pesem, 1)
            # cos^2 of all 8 banks -> sq[:, :, 0, :] (as [128, 4, 2, 101])
            nc.scalar.activation(
                sq[:, :, 0, :].rearrange("p m (two x) -> p m two x", two=2),
                ps[:, :, :, 0:NGP],
                mybir.ActivationFunctionType.Square,
            ).then_inc(actsem, 1)
            nc.scalar.activation(
                sq[:, :, 1, :].rearrange("p m (two x) -> p m two x", two=2),
                ps[:, :, :, NGP:NBH],
                mybir.ActivationFunctionType.Square,
            ).then_inc(actsem, 1)

        @block.vector
        def _(vector):
            vector.wait_ge(msem, 16)
            # masked squares per parity block (read only ACT/DMA data)
            vector.wait_ge(actsem, 1)
            nc.vector.tensor_mul(
                msq[:, :, 0], sq[:, :, 0], masksb[:, :, 0]
            )
            vector.wait_ge(actsem, 2)
            last_mul = nc.vector.tensor_mul(
                msq[:, :, 1], sq[:, :, 1], masksb[:, :, 1]
            )
            # totals read only ACT data -> safe before the drain
            nc.vector.tensor_reduce(
                outsb[:, :, 0],
                sq[:].rearrange("p m two x -> p m (two x)"),
                axis=mybir.AxisListType.X,
                op=mybir.AluOpType.add,
            ).then_inc(dvesem, 1)
            # drain DVE pipeline before reading its own msq outputs
            last_mul.then_inc(dv2, 1)
            vector.wait_ge(dv2, 1)
            nc.vector.tensor_reduce(
                outsb[:, :, 1],
                msq[:].rearrange("p m two x -> p m (two x)"),
                axis=mybir.AxisListType.X,
                op=mybir.AluOpType.add,
            ).then_inc(dvesem, 1)

    # Run Bacc's compile passes (register allocation, 1-wait legalization
    # via event-semaphore chains) — the PJRT exec path doesn't finalize.
    nc.finalize()
    return nc


def _host_prep(x, f_true_bpm, fs, delta_bpm, sampling_bpm, fmin_bpm, fmax_bpm):
    fs = int(fs)
    delta = int(delta_bpm)
    samp = int(sampling_bpm)
    fmin = int(fmin_bpm)
    fmax = int(fmax_bpm)

    n_grid = (fmax - fmin) // samp + 1
    assert n_grid == NG and fs == 30 and samp == 1, (n_grid, fs, samp)
    grid_bpm = fmin + samp * np.arange(n_grid, dtype=np.int64)
    ge = grid_bpm[grid_bpm % 2 == 0]  # 101 even bins
    go = grid_bpm[grid_bpm % 2 == 1]  # 100 odd bins

    # Folded basis over tau in [0, 900): theta = 2*pi*g*tau/1800.
    tau = np.arange(TF, dtype=np.float64)
    thE = 2.0 * np.pi * ge[:, None] * tau[None, :] / 1800.0  # [101, 900]
    thO = 2.0 * np.pi * go[:, None] * tau[None, :] / 1800.0  # [100, 900]
    basis = np.zeros((TF, 2 * NBH), dtype=BF16)
    basis[:, 0:NGE] = np.cos(thE).T.astype(BF16)
    basis[:, NGP : NGP + NGE] = np.sin(thE).T.astype(BF16)
    basis[:, NBH : NBH + NGO] = np.cos(thO).T.astype(BF16)
    basis[:, NBH + NGP : NBH + NGP + NGO] = np.sin(thO).T.astype(BF16)

    # Fold x: 4 segments of 900; even g sums plain, odd g alternates.
    s = x.astype(np.float64).reshape(B, 4, TF)
    xe = (s[:, 0] + s[:, 1] + s[:, 2] + s[:, 3]).astype(BF16)  # [B, 900]
    xo = (s[:, 0] - s[:, 1] + s[:, 2] - s[:, 3]).astype(BF16)

    # Wanted-band masks in [maskE(101) | maskO(100) pad] layout, doubled so
    # one elementwise multiply covers both (cos^2, sin^2) blocks.
    f64 = f_true_bpm.astype(np.int64)
    mask = np.zeros((B, 2, NBH), dtype=BF16)
    mask[:, 0, 0:NGE] = np.abs(ge[None, :] - f64[:, None]) <= delta
    mask[:, 0, NGP : NGP + NGO] = np.abs(go[None, :] - f64[:, None]) <= delta
    mask[:, 1, :] = mask[:, 0, :]
    mask = mask.reshape(B, 2 * NBH)

    in_maps = []
    for c in range(NCORES):
        sl = slice(c * BS, (c + 1) * BS)
        xbp = np.empty((TF, PCOL), dtype=BF16)
        xbp[:, 0:BS] = xe[sl].T
        xbp[:, BS:XCOL] = xo[sl].T
        xbp[:, XCOL:] = basis
        in_maps.append(
            {"xb": xbp, "mask": np.ascontiguousarray(mask[sl])}
        )

    n_wanted = 2 * delta // samp + 1
    n_unwanted = n_grid - n_wanted
    return in_maps, n_wanted, n_unwanted


def _finalize(outs, n_wanted, n_unwanted):
    # outs: list of [BS, 2] fp32 per core with (total_sum, wanted_sum) rows.
    full = np.concatenate(outs, axis=0).astype(np.float64)  # [B, 2]
    total, wanted = full[:, 0], full[:, 1]
    term1 = wanted / n_wanted
    term2 = (total - wanted) / n_unwanted
    snr = 10.0 * np.log10(term1 / term2)
    return np.array(-snr.mean(), dtype=np.float32)


def kernel(x, f_true_bpm, fs, delta_bpm, sampling_bpm, fmin_bpm, fmax_bpm):
    from concourse.bass_utils import run_bass_kernel_spmd

    x = np.asarray(x, dtype=np.float32)
    f_true_bpm = np.asarray(f_true_bpm)
    in_maps, n_wanted, n_unwanted = _host_prep(
        x, f_true_bpm, fs, delta_bpm, sampling_bpm, fmin_bpm, fmax_bpm
    )
    nc = _build_program()
    res = run_bass_kernel_spmd(nc, in_maps, core_ids=list(range(NCORES)))
    outs = [r["out"] for r in res.results]
    return _finalize(outs, n_wanted, n_unwanted)
